# revision 15
# baseline (speedup 1.0000x reference)
"""Multi-head causal attention (B=2, S=2048, C=2048, H=16) on 8 NeuronCores.

Sharding: 2-way data parallel over batch x 4-way tensor parallel over heads.
Core i handles batch b = i // 4 and heads [4*(i%4), 4*(i%4)+4)  (local
channel slice m in [512*(i%4), 512*(i%4)+512)).

Per-core kernel (single tile-pool scope; the tile scheduler interleaves the
three phases so projection matmuls of block sb+1 fill the exp-latency
bubbles of attention block sb):
  prologue: PE prewarm matmuls (HAM un-throttle during the initial DMA
            window) + dummy exp (ACT table load off the critical path)
  phase A (per 512-s-block): Qt/Kt (d on partitions) and V (s on
            partitions) projections in fp16 (fp32 PSUM); one PSUM bank per
            output slice, accumulated c-inner so only 2 'acc' banks rotate
  phase B (per 512-s-block, per head): transposed score tiles (t on
            partitions, s free) fp16 matmul, exp on ACT -> bf16, causal
            mask via gpsimd affine_select, AV + ones-matrix denominator
            matmuls in bf16 (denominator replicated on all partitions),
            normalize via DVE reciprocal_approx_fast + multiply
  phase C (per 512-s-block): local output projection partial + bo/4;
            partials of the 4 cores in a batch group are summed on the host
"""

import numpy as np

B, S, C, H = 2, 2048, 2048, 16
D = C // H            # 128 per-head dim
HL = 4                # heads per core
ML = HL * D           # 512 local channels
P = 128
SCALE = 1.0 / float(np.sqrt(D))

_CACHE = {}


def _build():
    import concourse.bacc as bacc
    import concourse.mybir as mybir
    import concourse.tile as tile

    f32 = mybir.dt.float32
    f32r = mybir.dt.float32r
    bf16 = mybir.dt.bfloat16
    f16 = mybir.dt.float16
    Exp = mybir.ActivationFunctionType.Exp
    is_ge = mybir.AluOpType.is_ge
    add = mybir.AluOpType.add
    mult = mybir.AluOpType.mult

    nc = bacc.Bacc("TRN2", target_bir_lowering=False, debug=False, num_devices=8)

    xt = nc.dram_tensor("xt", [C, S], f16, kind="ExternalInput")       # x[b].T
    wqt = nc.dram_tensor("wqt", [C, ML], f16, kind="ExternalInput")    # Wq.T cols
    wkt = nc.dram_tensor("wkt", [C, ML], f16, kind="ExternalInput")
    wvt = nc.dram_tensor("wvt", [C, ML], f16, kind="ExternalInput")
    wot = nc.dram_tensor("wot", [ML, C], f16, kind="ExternalInput")    # Wo.T rows
    bq = nc.dram_tensor("bq", [ML], f32, kind="ExternalInput")
    bk = nc.dram_tensor("bk", [ML], f32, kind="ExternalInput")
    bv = nc.dram_tensor("bv", [ML], f32, kind="ExternalInput")
    bo4 = nc.dram_tensor("bo4", [C], f32, kind="ExternalInput")        # bo / 4
    out = nc.dram_tensor("out", [S, C], f32, kind="ExternalOutput")

    NT = S // P    # 16 t/s tiles of 128
    NCT = C // P   # 16 contraction tiles

    with tile.TileContext(nc) as tc:
        with tc.tile_pool(name="persist", bufs=1) as pp_, \
             tc.tile_pool(name="work", bufs=1) as wk, \
             tc.tile_pool(name="ps", bufs=1, space="PSUM") as psp:
            Q = [pp_.tile([P, S], f16, tag=f"q{m}", name=f"q{m}") for m in range(HL)]
            K = [pp_.tile([P, S], f16, tag=f"k{m}", name=f"k{m}") for m in range(HL)]
            V = [pp_.tile([P, ML], bf16, tag=f"v{t}", name=f"v{t}") for t in range(NT)]
            WO = [pp_.tile([P, C], f16, tag=f"wo{m}", name=f"wo{m}") for m in range(HL)]
            ones_mat = pp_.tile([P, P], bf16, tag="ones_mat", name="ones_mat")
            ones_r = pp_.tile([P, P], f32, tag="ones_r", name="ones_r")
            warm = pp_.tile([P, 512], bf16, tag="warm", name="warm")
            bv_row = pp_.tile([1, ML], f32, tag="bv_row", name="bv_row")
            bo_row = pp_.tile([1, C], f32, tag="bo_row", name="bo_row")
            bv_bc = pp_.tile([P, ML], f32, tag="bv_bc", name="bv_bc")
            bo_bc = pp_.tile([P, C], f32, tag="bo_bc", name="bo_bc")
            bq_t = [pp_.tile([P, 1], f32, tag=f"bq{m}", name=f"bq{m}") for m in range(HL)]
            bk_t = [pp_.tile([P, 1], f32, tag=f"bk{m}", name=f"bk{m}") for m in range(HL)]

            # ---------------- prologue ----------------
            nc.gpsimd.memset(ones_mat[:], 1.0)
            nc.gpsimd.memset(ones_r[:], 1.0)
            nc.gpsimd.memset(warm[:], 0.0)
            # small bias loads on the gpsimd queue; bulk loads ordered by
            # first use on the sync queue (in-order issue per engine)
            nc.gpsimd.dma_start(bv_row[:], bv[None, :])
            nc.gpsimd.dma_start(bo_row[:], bo4[None, :])
            for m in range(HL):
                nc.gpsimd.dma_start(bq_t[m][:], bq[m * P:(m + 1) * P, None])
                nc.gpsimd.dma_start(bk_t[m][:], bk[m * P:(m + 1) * P, None])
            nc.gpsimd.partition_broadcast(bv_bc[:], bv_row[:])
            nc.gpsimd.partition_broadcast(bo_bc[:], bo_row[:])

            # PE prewarm: one dense accumulation group (back-to-back streams,
            # no per-matmul drain wait) keeps the PE busy through the HAM
            # SHORT window while the first input DMAs land
            pw = psp.tile([P, 512], f32, tag="mm", bufs=4, name="pw")
            for kk in range(20):
                nc.tensor.matmul(pw[:], ones_mat[:], warm[:],
                                 start=(kk == 0), stop=(kk == 19))
            # dummy exp: hoists the ~2.7us ACT table load off phase B
            dume = wk.tile([P, 32], bf16, tag="dume", name="dume")
            nc.scalar.activation(dume[:], warm[:, 0:32], Exp)

            # bulk weight loads, stream-major: each projection's accumulation
            # groups consume their weight stream c=0..15 in order, so the
            # streams must arrive whole, in first-use order (wv+x, wq, wk)
            wq_c, wk_c, wv_c = [], [], []
            xt_t0 = []
            for c in range(NCT):
                t = wk.tile([P, ML], f16, tag=f"cv{c}", name=f"cv{c}")
                nc.sync.dma_start(t[:], wvt[c * P:(c + 1) * P, :])
                wv_c.append(t)
                xti = wk.tile([P, 512], f16, tag=f"xt{c}", bufs=2,
                              name=f"xt{c}_0")
                nc.sync.dma_start(xti[:], xt[c * P:(c + 1) * P, 0:512])
                xt_t0.append(xti)
            for c in range(NCT):
                t = wk.tile([P, ML], f16, tag=f"cq{c}", name=f"cq{c}")
                nc.sync.dma_start(t[:], wqt[c * P:(c + 1) * P, :])
                wq_c.append(t)
            for c in range(NCT):
                t = wk.tile([P, ML], f16, tag=f"ck{c}", name=f"ck{c}")
                nc.sync.dma_start(t[:], wkt[c * P:(c + 1) * P, :])
                wk_c.append(t)
            for m in range(HL):
                nc.gpsimd.dma_start(WO[m][:], wot[m * P:(m + 1) * P, :])

            # ---------------- phase A (per s-block) ----------------
            def emit_A(sb):
                s0 = sb * 512
                if sb == 0:
                    xt_t = xt_t0
                else:
                    xt_t = []
                    for c in range(NCT):
                        xti = wk.tile([P, 512], f16, tag=f"xt{c}", bufs=2,
                                      name=f"xt{c}_{sb}")
                        nc.sync.dma_start(
                            xti[:], xt[c * P:(c + 1) * P, s0:s0 + 512])
                        xt_t.append(xti)

                # V projection (s on partitions); one PSUM bank per slice
                for i in range(4):
                    pv = psp.tile([P, ML], f32, tag="acc", bufs=2,
                                  name=f"pv{sb}_{i}")
                    for c in range(NCT):
                        nc.tensor.matmul(
                            pv[:], xt_t[c][:, i * P:(i + 1) * P], wv_c[c][:],
                            start=(c == 0), stop=(c == NCT - 1))
                    nc.vector.tensor_add(V[sb * 4 + i][:], pv[:], bv_bc[:])

                # Q then K projections (d on partitions)
                for proj in range(2):
                    w_c = wq_c if proj == 0 else wk_c
                    for m in range(HL):
                        pa = psp.tile([P, 512], f32, tag="acc", bufs=2,
                                      name=f"pa{sb}_{proj}{m}")
                        for c in range(NCT):
                            nc.tensor.matmul(
                                pa[:], w_c[c][:, m * P:(m + 1) * P],
                                xt_t[c][:],
                                start=(c == 0), stop=(c == NCT - 1))
                        if proj == 0:
                            nc.vector.tensor_scalar(
                                Q[m][:, s0:s0 + 512], pa[:],
                                bq_t[m][:], SCALE, add, mult)
                        else:
                            nc.vector.tensor_scalar_add(
                                K[m][:, s0:s0 + 512], pa[:], bk_t[m][:])

            # ---------------- phase B (per s-block, per head) ----------------
            def emit_B(sb):
                s0 = sb * 512
                ntile = 4 * (sb + 1)
                ndiag = 4 * sb       # tiles below the diagonal block row
                ot = []
                for h in range(HL):
                    po = psp.tile([P, 512], f32, tag="po", bufs=1,
                                  name=f"po{sb}{h}")
                    pd = psp.tile([P, 512], f32, tag="pd", bufs=1,
                                  name=f"pd{sb}{h}")
                    for ti in range(ntile):
                        # diagonal tiles: scores only needed for
                        # s >= ti*128 -> trim the free dim to W
                        off = max(0, ti * P - s0)
                        W = 512 - off
                        ps = psp.tile([P, 512], f32, tag="mm", bufs=4,
                                      name=f"ps{sb}{h}{ti}")
                        nc.tensor.matmul(ps[:, :W],
                                         K[h][:, ti * P:(ti + 1) * P],
                                         Q[h][:, s0 + off:s0 + 512],
                                         start=True, stop=True)
                        pe = wk.tile([P, 512], bf16, tag="pe", bufs=8,
                                     name=f"pe{sb}{h}{ti}")
                        nc.scalar.activation(pe[:, :W], ps[:, :W], Exp)
                        if ti >= ndiag:  # diagonal block row: causal mask
                            nc.gpsimd.affine_select(
                                out=pe[:, :W], in_=pe[:, :W],
                                compare_op=is_ge,
                                fill=0.0, base=0,
                                pattern=[[1, W]], channel_multiplier=-1)
                        nc.tensor.matmul(pd[:, off:512], ones_mat[:],
                                         pe[:, :W],
                                         start=(ti == 0),
                                         stop=(ti == ntile - 1))
                        nc.tensor.matmul(po[:, off:512],
                                         V[ti][:, h * P:(h + 1) * P],
                                         pe[:, :W], start=(ti == 0),
                                         stop=(ti == ntile - 1))
                    rec = wk.tile([P, 512], f32, tag="rec", bufs=2,
                                  name=f"rec{sb}{h}")
                    nc.vector.reciprocal_approx_fast(rec[:], pd[:])
                    oth = wk.tile([P, 512], f16, tag=f"ot{h}", bufs=2,
                                  name=f"ot{sb}{h}")
                    nc.vector.tensor_mul(oth[:], po[:], rec[:])
                    ot.append(oth)
                return ot

            # ---------------- phase C (per s-block) ----------------
            def emit_C(sb, ot):
                s0 = sb * 512
                for jb in range(4):
                    j0 = jb * 512
                    for st in range(4):
                        pp = psp.tile([P, 512], f32, tag="mm", bufs=4,
                                      name=f"pp{sb}{jb}{st}")
                        for m in range(HL):
                            nc.tensor.matmul(pp[:],
                                             ot[m][:, st * P:(st + 1) * P],
                                             WO[m][:, j0:j0 + 512],
                                             start=(m == 0), stop=(m == HL - 1))
                        outt = wk.tile([P, 512], f32, tag="outt", bufs=3,
                                       name=f"outt{sb}{jb}{st}")
                        nc.vector.tensor_add(outt[:], pp[:],
                                             bo_bc[:, j0:j0 + 512])
                        nc.sync.dma_start(
                            out[s0 + st * P:s0 + (st + 1) * P, j0:j0 + 512],
                            outt[:])

            emit_A(0)
            for sb in range(4):
                ot = emit_B(sb)
                emit_C(sb, ot)
                if sb + 1 < 4:
                    emit_A(sb + 1)

    nc.compile()
    return nc


def _get_program():
    if "nc" not in _CACHE:
        _CACHE["nc"] = _build()
    return _CACHE["nc"]


def make_in_maps(x, Wq, bq, Wk, bk, Wv, bv, Wo, bo):
    xtb = [np.ascontiguousarray(x[b].T).astype(np.float16) for b in range(B)]
    WqT = np.ascontiguousarray(Wq.T).astype(np.float16)
    WkT = np.ascontiguousarray(Wk.T).astype(np.float16)
    WvT = np.ascontiguousarray(Wv.T).astype(np.float16)
    WoT = np.ascontiguousarray(Wo.T).astype(np.float16)
    bo4 = (bo * 0.25).astype(np.float32)
    in_maps = []
    for core in range(8):
        b, hg = divmod(core, 4)
        ms = slice(hg * ML, (hg + 1) * ML)
        in_maps.append({
            "xt": xtb[b],
            "wqt": np.ascontiguousarray(WqT[:, ms]),
            "wkt": np.ascontiguousarray(WkT[:, ms]),
            "wvt": np.ascontiguousarray(WvT[:, ms]),
            "wot": np.ascontiguousarray(WoT[ms, :]),
            "bq": np.ascontiguousarray(bq[ms]),
            "bk": np.ascontiguousarray(bk[ms]),
            "bv": np.ascontiguousarray(bv[ms]),
            "bo4": bo4,
        })
    return in_maps


def run(inputs, trace=False):
    from concourse.bass_utils import run_bass_kernel_spmd

    nc = _get_program()
    in_maps = make_in_maps(
        inputs["x"], inputs["Wq"], inputs["bq"], inputs["Wk"], inputs["bk"],
        inputs["Wv"], inputs["bv"], inputs["Wo"], inputs["bo"])
    res = run_bass_kernel_spmd(nc, in_maps, core_ids=list(range(8)), trace=trace)
    partials = [np.asarray(res.results[c]["out"]) for c in range(8)]
    full = np.empty((B, S, C), dtype=np.float32)
    for b in range(B):
        acc = np.sum(np.stack(partials[4 * b:4 * b + 4], 0), 0,
                     dtype=np.float64)
        full[b] = acc.astype(np.float32)
    return full, res


def kernel(**inputs):
    full, _ = run(inputs, trace=False)
    return full


# revision 23
# speedup vs baseline: 1.1474x; 1.1474x over previous
"""Multi-head causal attention (B=2, S=2048, C=2048, H=16) on 8 NeuronCores.

Sharding: 2-way data parallel over batch x 4-way tensor parallel over heads.
Core i handles batch b = i // 4 and heads [4*(i%4), 4*(i%4)+4)  (local
channel slice m in [512*(i%4), 512*(i%4)+512)).

Per-core kernel (single tile-pool scope; the tile scheduler interleaves the
three phases so projection matmuls of block sb+1 fill the exp-latency
bubbles of attention block sb):
  prologue: PE prewarm matmuls (HAM un-throttle during the initial DMA
            window) + dummy exp (ACT table load off the critical path)
  phase A (per 512-s-block): Qt/Kt (d on partitions) and V (s on
            partitions) projections in fp16 (fp32 PSUM); one PSUM bank per
            output slice, accumulated c-inner so only 2 'acc' banks rotate
  phase B (per 512-s-block, per head): transposed score tiles (t on
            partitions, s free) fp16 matmul, exp on ACT -> bf16, causal
            mask via gpsimd affine_select, AV + ones-matrix denominator
            matmuls in bf16 (denominator replicated on all partitions),
            normalize via DVE reciprocal_approx_fast + multiply
  phase C (per 512-s-block): local output projection partial + bo/4;
            partials of the 4 cores in a batch group are summed on the host
"""

import numpy as np

B, S, C, H = 2, 2048, 2048, 16
D = C // H            # 128 per-head dim
HL = 4                # heads per core
ML = HL * D           # 512 local channels
P = 128
SCALE = 1.0 / float(np.sqrt(D))

_CACHE = {}


def _build():
    import concourse.bacc as bacc
    import concourse.mybir as mybir
    import concourse.tile as tile

    f32 = mybir.dt.float32
    f32r = mybir.dt.float32r
    bf16 = mybir.dt.bfloat16
    f16 = mybir.dt.float16
    Exp = mybir.ActivationFunctionType.Exp
    is_ge = mybir.AluOpType.is_ge
    add = mybir.AluOpType.add
    mult = mybir.AluOpType.mult

    nc = bacc.Bacc("TRN2", target_bir_lowering=False, debug=False, num_devices=8)

    xt = nc.dram_tensor("xt", [C, S], f16, kind="ExternalInput")       # x[b].T
    # [Wv.T | Wq.T | Wk.T] local column slices packed so each c-row loads
    # as one 3KB-per-partition DMA (batched transfers; see make_in_maps)
    wall = nc.dram_tensor("wall", [C, 3 * ML], f16, kind="ExternalInput")
    wot = nc.dram_tensor("wot", [ML, C], f16, kind="ExternalInput")    # Wo.T rows
    bq = nc.dram_tensor("bq", [ML], f32, kind="ExternalInput")
    bk = nc.dram_tensor("bk", [ML], f32, kind="ExternalInput")
    bv = nc.dram_tensor("bv", [ML], f32, kind="ExternalInput")
    bo4 = nc.dram_tensor("bo4", [C], f32, kind="ExternalInput")        # bo / 4
    out = nc.dram_tensor("out", [S, C], f32, kind="ExternalOutput")

    NT = S // P    # 16 t/s tiles of 128
    NCT = C // P   # 16 contraction tiles

    with tile.TileContext(nc) as tc:
        with tc.tile_pool(name="persist", bufs=1) as pp_, \
             tc.tile_pool(name="work", bufs=1) as wk, \
             tc.tile_pool(name="ps", bufs=1, space="PSUM") as psp:
            Q = [pp_.tile([P, S], f16, tag=f"q{m}", name=f"q{m}") for m in range(HL)]
            K = [pp_.tile([P, S], f16, tag=f"k{m}", name=f"k{m}") for m in range(HL)]
            V = [pp_.tile([P, ML], bf16, tag=f"v{t}", name=f"v{t}") for t in range(NT)]
            WO = [pp_.tile([P, C], f16, tag=f"wo{m}", name=f"wo{m}") for m in range(HL)]
            ones_mat = pp_.tile([P, P], bf16, tag="ones_mat", name="ones_mat")
            warm = pp_.tile([P, 512], bf16, tag="warm", name="warm")
            bv_row = pp_.tile([1, ML], f32, tag="bv_row", name="bv_row")
            bo_row = pp_.tile([1, C], f32, tag="bo_row", name="bo_row")
            bv_bc = pp_.tile([P, ML], f32, tag="bv_bc", name="bv_bc")
            bo_bc = pp_.tile([P, C], f32, tag="bo_bc", name="bo_bc")
            bq_t = [pp_.tile([P, 1], f32, tag=f"bq{m}", name=f"bq{m}") for m in range(HL)]
            bk_t = [pp_.tile([P, 1], f32, tag=f"bk{m}", name=f"bk{m}") for m in range(HL)]

            # ---------------- prologue ----------------
            nc.gpsimd.memset(ones_mat[:], 1.0)
            nc.gpsimd.memset(warm[:], 0.0)
            # small bias loads on the gpsimd queue; bulk loads ordered by
            # first use on the sync queue (in-order issue per engine)
            nc.gpsimd.dma_start(bv_row[:], bv[None, :])
            nc.gpsimd.dma_start(bo_row[:], bo4[None, :])
            for m in range(HL):
                nc.gpsimd.dma_start(bq_t[m][:], bq[m * P:(m + 1) * P, None])
                nc.gpsimd.dma_start(bk_t[m][:], bk[m * P:(m + 1) * P, None])
            nc.gpsimd.partition_broadcast(bv_bc[:], bv_row[:])
            nc.gpsimd.partition_broadcast(bo_bc[:], bo_row[:])

            # PE prewarm: one dense accumulation group (back-to-back streams,
            # no per-matmul drain wait) keeps the PE busy through the HAM
            # SHORT window while the first input DMAs land
            pw = psp.tile([P, 512], f32, tag="mm", bufs=4, name="pw")
            for kk in range(20):
                nc.tensor.matmul(pw[:], ones_mat[:], warm[:],
                                 start=(kk == 0), stop=(kk == 19))
            # dummy exp: hoists the ~2.7us ACT table load off phase B
            dume = wk.tile([P, 32], bf16, tag="dume", name="dume")
            nc.scalar.activation(dume[:], warm[:, 0:32], Exp)

            # bulk weight loads: one 3KB-per-partition DMA per c-row brings
            # wv|wq|wk together (batched, descriptor-efficient)
            cw = []
            for c in range(NCT):
                t = wk.tile([P, 3 * ML], f16, tag=f"cw{c}", name=f"cw{c}")
                nc.sync.dma_start(t[:], wall[c * P:(c + 1) * P, :])
                cw.append(t)
            # x tiles arrive as 1024-wide s-block pairs (2KB lines)
            xt_t0 = []
            for c in range(NCT):
                xti = wk.tile([P, 1024], f16, tag=f"xt{c}", bufs=1,
                              name=f"xt{c}_p0")
                nc.sync.dma_start(xti[:], xt[c * P:(c + 1) * P, 0:1024])
                xt_t0.append(xti)
            for m in range(HL):
                nc.gpsimd.dma_start(WO[m][:], wot[m * P:(m + 1) * P, :])

            # ---------------- phase A (per s-block) ----------------
            xt_pair = {0: xt_t0}

            def emit_A(sb):
                s0 = sb * 512
                pair, base = divmod(s0, 1024)
                if pair not in xt_pair:
                    xt_t = []
                    for c in range(NCT):
                        xti = wk.tile([P, 1024], f16, tag=f"xt{c}", bufs=1,
                                      name=f"xt{c}_p{pair}")
                        nc.sync.dma_start(
                            xti[:], xt[c * P:(c + 1) * P,
                                       pair * 1024:(pair + 1) * 1024])
                        xt_t.append(xti)
                    xt_pair[pair] = xt_t
                xt_t = xt_pair[pair]

                # V projection (s on partitions); one PSUM bank per slice
                for i in range(4):
                    pv = psp.tile([P, ML], f32, tag="acc", bufs=2,
                                  name=f"pv{sb}_{i}")
                    for c in range(NCT):
                        nc.tensor.matmul(
                            pv[:],
                            xt_t[c][:, base + i * P:base + (i + 1) * P],
                            cw[c][:, 0:ML],
                            start=(c == 0), stop=(c == NCT - 1))
                    nc.vector.tensor_add(V[sb * 4 + i][:], pv[:], bv_bc[:])

                # Q then K projections (d on partitions)
                for proj in range(2):
                    woff = ML if proj == 0 else 2 * ML
                    for m in range(HL):
                        pa = psp.tile([P, 512], f32, tag="acc", bufs=2,
                                      name=f"pa{sb}_{proj}{m}")
                        for c in range(NCT):
                            nc.tensor.matmul(
                                pa[:],
                                cw[c][:, woff + m * P:woff + (m + 1) * P],
                                xt_t[c][:, base:base + 512],
                                start=(c == 0), stop=(c == NCT - 1))
                        if proj == 0:
                            nc.vector.tensor_scalar(
                                Q[m][:, s0:s0 + 512], pa[:],
                                bq_t[m][:], SCALE, add, mult)
                        else:
                            nc.vector.tensor_scalar_add(
                                K[m][:, s0:s0 + 512], pa[:], bk_t[m][:])

            # ---------------- phase B (per s-block, per head) ----------------
            def emit_B(sb):
                s0 = sb * 512
                ntile = 4 * (sb + 1)
                ndiag = 4 * sb       # tiles below the diagonal block row
                ot = []
                for h in range(HL):
                    po = psp.tile([P, 512], f32, tag="po", bufs=1,
                                  name=f"po{sb}{h}")
                    pd = psp.tile([P, 512], f32, tag="pd", bufs=1,
                                  name=f"pd{sb}{h}")
                    for ti in range(ntile):
                        # diagonal tiles: scores only needed for
                        # s >= ti*128 -> trim the free dim to W
                        off = max(0, ti * P - s0)
                        W = 512 - off
                        ps = psp.tile([P, 512], f32, tag="mm", bufs=4,
                                      name=f"ps{sb}{h}{ti}")
                        nc.tensor.matmul(ps[:, :W],
                                         K[h][:, ti * P:(ti + 1) * P],
                                         Q[h][:, s0 + off:s0 + 512],
                                         start=True, stop=True)
                        pe = wk.tile([P, 512], bf16, tag="pe", bufs=8,
                                     name=f"pe{sb}{h}{ti}")
                        nc.scalar.activation(pe[:, :W], ps[:, :W], Exp)
                        if ti >= ndiag:  # diagonal block row: causal mask
                            nc.gpsimd.affine_select(
                                out=pe[:, :W], in_=pe[:, :W],
                                compare_op=is_ge,
                                fill=0.0, base=0,
                                pattern=[[1, W]], channel_multiplier=-1)
                        nc.tensor.matmul(pd[:, off:512], ones_mat[:],
                                         pe[:, :W],
                                         start=(ti == 0),
                                         stop=(ti == ntile - 1))
                        nc.tensor.matmul(po[:, off:512],
                                         V[ti][:, h * P:(h + 1) * P],
                                         pe[:, :W], start=(ti == 0),
                                         stop=(ti == ntile - 1))
                    rec = wk.tile([P, 512], f32, tag="rec", bufs=2,
                                  name=f"rec{sb}{h}")
                    nc.vector.reciprocal_approx_fast(rec[:], pd[:])
                    oth = wk.tile([P, 512], f16, tag=f"ot{h}", bufs=2,
                                  name=f"ot{sb}{h}")
                    nc.vector.tensor_mul(oth[:], po[:], rec[:])
                    ot.append(oth)
                return ot

            # ---------------- phase C (per s-block) ----------------
            def emit_C(sb, ot):
                s0 = sb * 512
                for st in range(4):
                    outt = wk.tile([P, C], f32, tag="outt", bufs=2,
                                   name=f"outt{sb}{st}")
                    for jb in range(4):
                        j0 = jb * 512
                        pp = psp.tile([P, 512], f32, tag="mm", bufs=4,
                                      name=f"pp{sb}{jb}{st}")
                        for m in range(HL):
                            nc.tensor.matmul(pp[:],
                                             ot[m][:, st * P:(st + 1) * P],
                                             WO[m][:, j0:j0 + 512],
                                             start=(m == 0), stop=(m == HL - 1))
                        nc.vector.tensor_add(outt[:, j0:j0 + 512], pp[:],
                                             bo_bc[:, j0:j0 + 512])
                    # one batched 8KB-per-partition row store
                    nc.sync.dma_start(
                        out[s0 + st * P:s0 + (st + 1) * P, :], outt[:])

            emit_A(0)
            for sb in range(4):
                ot = emit_B(sb)
                emit_C(sb, ot)
                if sb + 1 < 4:
                    emit_A(sb + 1)

    nc.compile()
    return nc


def _get_program():
    if "nc" not in _CACHE:
        _CACHE["nc"] = _build()
    return _CACHE["nc"]


def make_in_maps(x, Wq, bq, Wk, bk, Wv, bv, Wo, bo):
    xtb = [np.ascontiguousarray(x[b].T).astype(np.float16) for b in range(B)]
    WqT = np.ascontiguousarray(Wq.T).astype(np.float16)
    WkT = np.ascontiguousarray(Wk.T).astype(np.float16)
    WvT = np.ascontiguousarray(Wv.T).astype(np.float16)
    WoT = np.ascontiguousarray(Wo.T).astype(np.float16)
    bo4 = (bo * 0.25).astype(np.float32)
    in_maps = []
    for core in range(8):
        b, hg = divmod(core, 4)
        ms = slice(hg * ML, (hg + 1) * ML)
        wall = np.ascontiguousarray(
            np.concatenate([WvT[:, ms], WqT[:, ms], WkT[:, ms]], axis=1))
        in_maps.append({
            "xt": xtb[b],
            "wall": wall,
            "wot": np.ascontiguousarray(WoT[ms, :]),
            "bq": np.ascontiguousarray(bq[ms]),
            "bk": np.ascontiguousarray(bk[ms]),
            "bv": np.ascontiguousarray(bv[ms]),
            "bo4": bo4,
        })
    return in_maps


def run(inputs, trace=False):
    from concourse.bass_utils import run_bass_kernel_spmd

    nc = _get_program()
    in_maps = make_in_maps(
        inputs["x"], inputs["Wq"], inputs["bq"], inputs["Wk"], inputs["bk"],
        inputs["Wv"], inputs["bv"], inputs["Wo"], inputs["bo"])
    res = run_bass_kernel_spmd(nc, in_maps, core_ids=list(range(8)), trace=trace)
    partials = [np.asarray(res.results[c]["out"]) for c in range(8)]
    full = np.empty((B, S, C), dtype=np.float32)
    for b in range(B):
        acc = np.sum(np.stack(partials[4 * b:4 * b + 4], 0), 0,
                     dtype=np.float64)
        full[b] = acc.astype(np.float32)
    return full, res


def kernel(**inputs):
    full, _ = run(inputs, trace=False)
    return full


# revision 46
# speedup vs baseline: 1.1832x; 1.0312x over previous
"""Multi-head causal attention (B=2, S=2048, C=2048, H=16) on 8 NeuronCores.

Sharding: 2-way data parallel over batch x 4-way tensor parallel over heads.
Core i handles batch b = i // 4 and heads [4*(i%4), 4*(i%4)+4)  (local
channel slice m in [512*(i%4), 512*(i%4)+512)).

Per-core kernel (single tile-pool scope; the tile scheduler interleaves the
three phases so projection matmuls of block sb+1 fill the exp-latency
bubbles of attention block sb):
  prologue: PE prewarm matmuls (HAM un-throttle during the initial DMA
            window) + dummy exp (ACT table load off the critical path)
  phase A (per 512-s-block): Qt/Kt (d on partitions) and V (s on
            partitions) projections in fp16 (fp32 PSUM); one PSUM bank per
            output slice, accumulated c-inner so only 2 'acc' banks rotate
  phase B (per 512-s-block, per head): transposed score tiles (t on
            partitions, s free) fp16 matmul, exp on ACT -> bf16, causal
            mask via gpsimd affine_select, AV + ones-matrix denominator
            matmuls in bf16 (denominator replicated on all partitions),
            normalize via DVE reciprocal_approx_fast + multiply
  phase C (per 512-s-block): local output projection partial + bo/4;
            partials of the 4 cores in a batch group are summed on the host
"""

import numpy as np

B, S, C, H = 2, 2048, 2048, 16
D = C // H            # 128 per-head dim
HL = 4                # heads per core
ML = HL * D           # 512 local channels
P = 128
SCALE = 1.0 / float(np.sqrt(D))

_CACHE = {}


def _build():
    import concourse.bacc as bacc
    import concourse.mybir as mybir
    import concourse.tile as tile

    f32 = mybir.dt.float32
    f32r = mybir.dt.float32r
    bf16 = mybir.dt.bfloat16
    f16 = mybir.dt.float16
    Exp = mybir.ActivationFunctionType.Exp
    is_ge = mybir.AluOpType.is_ge
    add = mybir.AluOpType.add
    mult = mybir.AluOpType.mult

    nc = bacc.Bacc("TRN2", target_bir_lowering=False, debug=False, num_devices=8)

    xt = nc.dram_tensor("xt", [C, S], f16, kind="ExternalInput")       # x[b].T
    wqt = nc.dram_tensor("wqt", [C, ML], f16, kind="ExternalInput")    # Wq.T cols
    wkt = nc.dram_tensor("wkt", [C, ML], f16, kind="ExternalInput")
    wvt = nc.dram_tensor("wvt", [C, ML], f16, kind="ExternalInput")
    wot = nc.dram_tensor("wot", [ML, C], f16, kind="ExternalInput")    # Wo.T rows
    bq = nc.dram_tensor("bq", [ML], f32, kind="ExternalInput")
    bk = nc.dram_tensor("bk", [ML], f32, kind="ExternalInput")
    bv = nc.dram_tensor("bv", [ML], f32, kind="ExternalInput")
    bo4 = nc.dram_tensor("bo4", [C], f32, kind="ExternalInput")        # bo / 4
    out = nc.dram_tensor("out", [S, C], f32, kind="ExternalOutput")

    NT = S // P    # 16 t/s tiles of 128
    NCT = C // P   # 16 contraction tiles

    with tile.TileContext(nc) as tc:
        with tc.tile_pool(name="persist", bufs=1) as pp_, \
             tc.tile_pool(name="work", bufs=1) as wk, \
             tc.tile_pool(name="ps", bufs=1, space="PSUM") as psp:
            Q = [pp_.tile([P, S], f16, tag=f"q{m}", name=f"q{m}") for m in range(HL)]
            K = [pp_.tile([P, S], f16, tag=f"k{m}", name=f"k{m}") for m in range(HL)]
            V = [pp_.tile([P, ML], bf16, tag=f"v{t}", name=f"v{t}") for t in range(NT)]
            WO = [pp_.tile([P, C], f16, tag=f"wo{m}", name=f"wo{m}") for m in range(HL)]
            ones_mat = pp_.tile([P, P], bf16, tag="ones_mat", name="ones_mat")
            warm = pp_.tile([P, 512], bf16, tag="warm", name="warm")
            bv_row = pp_.tile([1, ML], f32, tag="bv_row", name="bv_row")
            bo_row = pp_.tile([1, C], f32, tag="bo_row", name="bo_row")
            bv_bc = pp_.tile([P, ML], f32, tag="bv_bc", name="bv_bc")
            bo_bc = pp_.tile([P, C], f32, tag="bo_bc", name="bo_bc")
            bq_t = [pp_.tile([P, 1], f32, tag=f"bq{m}", name=f"bq{m}") for m in range(HL)]
            bk_t = [pp_.tile([P, 1], f32, tag=f"bk{m}", name=f"bk{m}") for m in range(HL)]

            # ---------------- prologue ----------------
            nc.gpsimd.memset(ones_mat[:], 1.0)
            nc.gpsimd.memset(warm[:], 0.0)
            # small bias loads on the gpsimd queue; bulk loads ordered by
            # first use on the sync queue (in-order issue per engine)
            nc.gpsimd.dma_start(bv_row[:], bv[None, :])
            nc.gpsimd.dma_start(bo_row[:], bo4[None, :])
            for m in range(HL):
                nc.gpsimd.dma_start(bq_t[m][:], bq[m * P:(m + 1) * P, None])
                nc.gpsimd.dma_start(bk_t[m][:], bk[m * P:(m + 1) * P, None])
            nc.gpsimd.partition_broadcast(bv_bc[:], bv_row[:])
            nc.gpsimd.partition_broadcast(bo_bc[:], bo_row[:])

            # PE prewarm: one dense accumulation group (back-to-back streams,
            # no per-matmul drain wait) keeps the PE busy through the HAM
            # SHORT window while the first input DMAs land
            pw = psp.tile([P, 512], f32, tag="mm", bufs=4, name="pw")
            for kk in range(12):
                nc.tensor.matmul(pw[:], ones_mat[:], warm[:],
                                 start=(kk == 0), stop=(kk == 11))
            # dummy exp: hoists the ~2.7us ACT table load off phase B
            dume = wk.tile([P, 32], bf16, tag="dume", name="dume")
            nc.scalar.activation(dume[:], warm[:, 0:32], Exp)

            # bulk weight loads, stream-major in first-use order (wv+x0
            # interleaved, then wq, then wk); descriptor issue (~0.6-1us per
            # dma_start, serial per sequencer) spread over three sequencers.
            # Block 0's projections chase this wavefront c-outer below.
            # c-major issue across all four input streams, matching block
            # 0's c-outer consumption waves below
            issuers = [nc.sync, nc.scalar, nc.gpsimd]
            wv_c, wq_c, wk_c = [], [], []
            xt_t0 = []
            for c in range(NCT):
                eng = issuers[c % 3]
                t = wk.tile([P, ML], f16, tag=f"cv{c}", name=f"cv{c}")
                eng.dma_start(t[:], wvt[c * P:(c + 1) * P, :])
                wv_c.append(t)
                xti = wk.tile([P, 512], f16, tag=f"xt{c}", bufs=2,
                              name=f"xt{c}_0")
                eng.dma_start(xti[:], xt[c * P:(c + 1) * P, 0:512])
                xt_t0.append(xti)
                t = wk.tile([P, ML], f16, tag=f"cq{c}", name=f"cq{c}")
                eng.dma_start(t[:], wqt[c * P:(c + 1) * P, :])
                wq_c.append(t)
                t = wk.tile([P, ML], f16, tag=f"ck{c}", name=f"ck{c}")
                eng.dma_start(t[:], wkt[c * P:(c + 1) * P, :])
                wk_c.append(t)

            # ---------------- phase A (per s-block) ----------------
            xt_blocks = {0: xt_t0}

            def emit_A(sb):
                s0 = sb * 512
                if sb not in xt_blocks:
                    xt_t = []
                    for c in range(NCT):
                        xti = wk.tile([P, 512], f16, tag=f"xt{c}", bufs=2,
                                      name=f"xt{c}_{sb}")
                        nc.sync.dma_start(
                            xti[:], xt[c * P:(c + 1) * P, s0:s0 + 512])
                        xt_t.append(xti)
                    xt_blocks[sb] = xt_t
                xt_t = xt_blocks[sb]

                # block 0 runs while the input wavefront lands: emit each
                # projection c-outer across 4 concurrent PSUM banks (acc+mm)
                # so the PE tracks tile arrival with no head-of-line block.
                # Later blocks have data on-chip; 2 rotating banks suffice.
                grp = [0]

                def acc_alloc(nm):
                    grp[0] += 1
                    if sb == 0 and grp[0] % 2 == 0:
                        return psp.tile([P, 512], f32, tag="mm", bufs=4,
                                        name=nm)
                    return psp.tile([P, 512], f32, tag="acc", bufs=2,
                                    name=nm)

                def drain_qk(proj, m, pa):
                    if proj == 0:
                        nc.vector.tensor_scalar(
                            Q[m][:, s0:s0 + 512], pa[:],
                            bq_t[m][:], SCALE, add, mult)
                    else:
                        nc.vector.tensor_scalar_add(
                            K[m][:, s0:s0 + 512], pa[:], bk_t[m][:])

                if sb == 0:
                    # all three weight streams arrive concurrently
                    # (round-robin DMA rings): consume them concurrently in
                    # two c-outer waves of 6 accumulation groups
                    for w in range(2):
                        i0 = 2 * w
                        pvs = [acc_alloc(f"pv0_{i0 + i}") for i in range(2)]
                        pqs = [acc_alloc(f"pa0_0{i0 + i}") for i in range(2)]
                        pks = [acc_alloc(f"pa0_1{i0 + i}") for i in range(2)]
                        for c in range(NCT):
                            st, sp = (c == 0), (c == NCT - 1)
                            for i in range(2):
                                nc.tensor.matmul(
                                    pvs[i][:],
                                    xt_t[c][:, (i0 + i) * P:(i0 + i + 1) * P],
                                    wv_c[c][:], start=st, stop=sp)
                            for i in range(2):
                                nc.tensor.matmul(
                                    pqs[i][:],
                                    wq_c[c][:, (i0 + i) * P:(i0 + i + 1) * P],
                                    xt_t[c][:], start=st, stop=sp)
                            for i in range(2):
                                nc.tensor.matmul(
                                    pks[i][:],
                                    wk_c[c][:, (i0 + i) * P:(i0 + i + 1) * P],
                                    xt_t[c][:], start=st, stop=sp)
                        for i in range(2):
                            nc.vector.tensor_add(V[i0 + i][:], pvs[i][:],
                                                 bv_bc[:])
                            drain_qk(0, i0 + i, pqs[i])
                            drain_qk(1, i0 + i, pks[i])
                else:
                    # data on-chip: 2 rotating banks suffice
                    for i in range(4):
                        pv = acc_alloc(f"pv{sb}_{i}")
                        for c in range(NCT):
                            nc.tensor.matmul(
                                pv[:], xt_t[c][:, i * P:(i + 1) * P],
                                wv_c[c][:],
                                start=(c == 0), stop=(c == NCT - 1))
                        nc.vector.tensor_add(V[sb * 4 + i][:], pv[:],
                                             bv_bc[:])
                    for proj in range(2):
                        w_c = wq_c if proj == 0 else wk_c
                        for m in range(HL):
                            pa = acc_alloc(f"pa{sb}_{proj}{m}")
                            for c in range(NCT):
                                nc.tensor.matmul(
                                    pa[:],
                                    w_c[c][:, m * P:(m + 1) * P], xt_t[c][:],
                                    start=(c == 0), stop=(c == NCT - 1))
                            drain_qk(proj, m, pa)

            # ---------------- phase B (per s-block, per head) ----------------
            def ps_alloc(sb, k, nm):
                # in the last block phase A is finished: borrow its idle
                # 'acc' banks to deepen the score/out-proj pipelines
                if sb == 3 and k % 3 == 2:
                    return psp.tile([P, 512], f32, tag="acc", bufs=2,
                                    name=nm)
                return psp.tile([P, 512], f32, tag="mm", bufs=4, name=nm)

            def emit_B(sb):
                s0 = sb * 512
                ntile = 4 * (sb + 1)
                ndiag = 4 * sb       # tiles below the diagonal block row
                ot = []
                for h in range(HL):
                    po = psp.tile([P, 512], f32, tag="po", bufs=1,
                                  name=f"po{sb}{h}")
                    pd = psp.tile([P, 512], f32, tag="pd", bufs=1,
                                  name=f"pd{sb}{h}")
                    for ti in range(ntile):
                        # diagonal tiles: scores only needed for
                        # s >= ti*128 -> trim the free dim to W
                        off = max(0, ti * P - s0)
                        W = 512 - off
                        ps = ps_alloc(sb, h * 16 + ti, f"ps{sb}{h}{ti}")
                        nc.tensor.matmul(ps[:, :W],
                                         K[h][:, ti * P:(ti + 1) * P],
                                         Q[h][:, s0 + off:s0 + 512],
                                         start=True, stop=True)
                        pe = wk.tile([P, 512], bf16, tag="pe", bufs=8,
                                     name=f"pe{sb}{h}{ti}")
                        nc.scalar.activation(pe[:, :W], ps[:, :W], Exp)
                        if ti >= ndiag:  # diagonal block row: causal mask
                            nc.gpsimd.affine_select(
                                out=pe[:, :W], in_=pe[:, :W],
                                compare_op=is_ge,
                                fill=0.0, base=0,
                                pattern=[[1, W]], channel_multiplier=-1)
                        nc.tensor.matmul(pd[:, off:512], ones_mat[:],
                                         pe[:, :W],
                                         start=(ti == 0),
                                         stop=(ti == ntile - 1))
                        nc.tensor.matmul(po[:, off:512],
                                         V[ti][:, h * P:(h + 1) * P],
                                         pe[:, :W], start=(ti == 0),
                                         stop=(ti == ntile - 1))
                    rec = wk.tile([P, 512], f32, tag="rec", bufs=2,
                                  name=f"rec{sb}{h}")
                    nc.vector.reciprocal_approx_fast(rec[:], pd[:])
                    oth = wk.tile([P, 512], f16, tag=f"ot{h}", bufs=2,
                                  name=f"ot{sb}{h}")
                    nc.vector.tensor_mul(oth[:], po[:], rec[:])
                    ot.append(oth)
                return ot

            # ---------------- phase C (per s-block) ----------------
            def emit_C(sb, ot):
                s0 = sb * 512
                for st in range(4):
                    outt = wk.tile([P, C], f32, tag="outt", bufs=2,
                                   name=f"outt{sb}{st}")
                    for jb in range(4):
                        j0 = jb * 512
                        pp = ps_alloc(sb, st * 4 + jb, f"pp{sb}{jb}{st}")
                        for m in range(HL):
                            nc.tensor.matmul(pp[:],
                                             ot[m][:, st * P:(st + 1) * P],
                                             WO[m][:, j0:j0 + 512],
                                             start=(m == 0), stop=(m == HL - 1))
                        nc.vector.tensor_add(outt[:, j0:j0 + 512], pp[:],
                                             bo_bc[:, j0:j0 + 512])
                    # one batched 8KB-per-partition row store
                    nc.sync.dma_start(
                        out[s0 + st * P:s0 + (st + 1) * P, :], outt[:])

            emit_A(0)
            # WO loads issued after block 0's input wavefront (first use
            # is phase C of block 0, ~100us in)
            for m in range(HL):
                nc.gpsimd.dma_start(WO[m][:], wot[m * P:(m + 1) * P, :])
            for sb in range(4):
                ot = emit_B(sb)
                emit_C(sb, ot)
                if sb + 1 < 4:
                    emit_A(sb + 1)

    nc.compile()
    return nc


def _get_program():
    if "nc" not in _CACHE:
        _CACHE["nc"] = _build()
    return _CACHE["nc"]


def make_in_maps(x, Wq, bq, Wk, bk, Wv, bv, Wo, bo):
    xtb = [np.ascontiguousarray(x[b].T).astype(np.float16) for b in range(B)]
    WqT = np.ascontiguousarray(Wq.T).astype(np.float16)
    WkT = np.ascontiguousarray(Wk.T).astype(np.float16)
    WvT = np.ascontiguousarray(Wv.T).astype(np.float16)
    WoT = np.ascontiguousarray(Wo.T).astype(np.float16)
    bo4 = (bo * 0.25).astype(np.float32)
    in_maps = []
    for core in range(8):
        b, hg = divmod(core, 4)
        ms = slice(hg * ML, (hg + 1) * ML)
        in_maps.append({
            "xt": xtb[b],
            "wqt": np.ascontiguousarray(WqT[:, ms]),
            "wkt": np.ascontiguousarray(WkT[:, ms]),
            "wvt": np.ascontiguousarray(WvT[:, ms]),
            "wot": np.ascontiguousarray(WoT[ms, :]),
            "bq": np.ascontiguousarray(bq[ms]),
            "bk": np.ascontiguousarray(bk[ms]),
            "bv": np.ascontiguousarray(bv[ms]),
            "bo4": bo4,
        })
    return in_maps


def run(inputs, trace=False):
    from concourse.bass_utils import run_bass_kernel_spmd

    nc = _get_program()
    in_maps = make_in_maps(
        inputs["x"], inputs["Wq"], inputs["bq"], inputs["Wk"], inputs["bk"],
        inputs["Wv"], inputs["bv"], inputs["Wo"], inputs["bo"])
    res = run_bass_kernel_spmd(nc, in_maps, core_ids=list(range(8)), trace=trace)
    partials = [np.asarray(res.results[c]["out"]) for c in range(8)]
    full = np.empty((B, S, C), dtype=np.float32)
    for b in range(B):
        acc = np.sum(np.stack(partials[4 * b:4 * b + 4], 0), 0,
                     dtype=np.float64)
        full[b] = acc.astype(np.float32)
    return full, res


def kernel(**inputs):
    full, _ = run(inputs, trace=False)
    return full
